# revision 1
# baseline (speedup 1.0000x reference)
"""DistMaps kernel for Trainium2 (Bass), SPMD over 8 NeuronCores.

Problem: out[b, 0, z, y, x] = 1.0 if min_p ((z-pz)^2 + (y-py)^2 + (x-px)^2) <= 25
over the 24 points p of batch b, else 0.0.  (The x input is only used for its
shape.)

Strategy
--------
The output is a union of radius-5 balls around 24 points per batch.  Shard the
volume over D (96 = 8 cores x 12 z-slices) and, per the point-parallel sharding
hint, also shard the POINTS: each core receives only the points whose +-5
z-window intersects its slab (~6 of 24, padded to NPTS=12 with far-away
dummies), so no collective is needed.

Per z-slice the plane is a union of disks; a disk is, row by row, an
x-interval: ind[y', x] = (x-px)^2 <= t - (y'-py)^2.  K-rows = (point, row
offset j in -4..5) give K = 10*NPTS <= 128 rows per batch covering every
integer y with |y-py| <= 5, and

  plane[y, x] = sum_k onehot[k, y] * ind_z[k, x]   (TensorE matmul, bf16->f32)
  out = plane > 0                                  (DVE is_gt / ScalarE Sign)

onehot[k, y] = (y == floor(py)+j) is z-invariant (floor via the fp32
magic-number round of py-0.5); ind_z is one tensor_scalar (DVE) or
relu-activation (ScalarE) per (slice, group).  Rows with negative thresholds
are all-zero automatically, so the program is static and SPMD-identical.

The main path is raw Bacc with five counting semaphores (no Tile scheduling
tail); a Tile-scheduled variant handles the NPTS > 12 fallback.
"""

import numpy as np

B = 2
D, H, W = 96, 160, 160
P = 24
J = 10
NCORES = 8
DLOC = D // NCORES   # 12
ZG = 3               # z-slices per matmul group
NZG = DLOC // ZG     # 4
R2 = 25.0
NWARM = 20
RG = None  # set per-build in the tile fallback

_prog_cache = {}


def _build_program_fast(npts):
    from contextlib import ExitStack

    import concourse.mybir as mybir
    from concourse import bacc

    f32 = mybir.dt.float32
    bf16 = mybir.dt.bfloat16
    op = mybir.AluOpType
    Act = mybir.ActivationFunctionType

    K = npts * J
    NRG = (K + 127) // 128
    NB = B * NRG
    rgs = [min(128, K - 128 * g) for g in range(NRG)]
    NW = ZG * W          # 480
    NW2 = 2 * NW         # 960, zgroup pair

    nc = bacc.Bacc(trn_type="TRN2")

    pts_d = nc.dram_tensor("pts", [128, 4 * NB], f32, kind="ExternalInput")
    zgb_d = nc.dram_tensor("zgridb", [128, DLOC], f32, kind="ExternalInput")
    xgb_d = nc.dram_tensor("xgridb", [128, W], f32, kind="ExternalInput")
    out_d = nc.dram_tensor("out", [B, H, DLOC, W], f32, kind="ExternalOutput")
    # remainder rows in device-blocked layout [b, (zg,yo), (zi x)]; host unshards
    out1_d = nc.dram_tensor("out1", [B, 128, ZG * W], f32, kind="ExternalOutput")

    s_in = nc.alloc_semaphore("s_in")
    s_dve = nc.alloc_semaphore("s_dve")
    s_act = nc.alloc_semaphore("s_act")
    s_pe = nc.alloc_semaphore("s_pe")
    s_st = nc.alloc_semaphore("s_st")
    sem_nums = [s.num for s in (s_in, s_dve, s_act, s_pe, s_st)]
    assert max(sem_nums) - min(sem_nums) + 1 == len(sem_nums), sem_nums
    sem_range = range(min(sem_nums), max(sem_nums) + 1)

    c = {"dve": 0, "act": 0, "pe": 0, "st": 0}

    if True:
        def sb(nm, shape, dt):
            return nc.alloc_sbuf_tensor(nm, shape, dt)

        def psm(nm, shape):
            return nc.alloc_psum_tensor(nm, shape, f32)

        xgb = sb("xgb", [128, W], f32)
        zgb = sb("zgb", [128, DLOC], f32)
        psb = sb("psb", [128, 4 * NB], f32)
        col = sb("col", [128, 4 * NB], f32)
        dzt = sb("dzt", [128, NB * DLOC], f32)
        tts = sb("tts", [128, NB * DLOC], f32)
        dx2 = sb("dx2", [128, NB * W], f32)
        oh = sb("oh", [128, NB * W], bf16)
        warm_a = sb("warm_a", [128, 512], bf16)
        rhs_t = [sb(f"rhs{i}", [128, NW], bf16) for i in range(B * NZG)]
        ob0 = [sb(f"ob0_{i}", [128, 1024], f32) for i in range(B * NZG // 2)]
        ob1 = [sb(f"ob1_{i}", [128, NW], f32) for i in range(B)]

        ps0 = [psm(f"ps0_{i}", [128, 1024]) for i in range(3)]  # 2 banks each
        ps1 = [psm(f"ps1_{i}", [128, NW]) for i in range(B)]    # 1 bank each
        warm_ps = ps1[1]  # warm-up scratch; rewritten by b1's real matmuls

        # ---- SP: input loads (psb first: the setup column chain only needs it) ----
        nc.sync.dma_start(
            out=psb[:, :], in_=pts_d[:, :], single_packet=True
        ).then_inc(s_in, 16)
        # grid loads on the scalar HWDGE ring so they issue in parallel with
        # psb on the sync ring.  They count on s_st (stores adjust their base)
        # so the psb wait (s_in>=16) cannot be satisfied out of order by a
        # grid load completing first on the other ring.
        nc.scalar.dma_start(out=zgb[:, :], in_=zgb_d[:, :]).then_inc(s_st, 16)
        nc.scalar.dma_start(out=xgb[:, :], in_=xgb_d[:, :]).then_inc(s_st, 16)
        c["st"] = 32

        # ACT: dummy activation early so the ~1.3us ACT_TABLE_LOAD that walrus
        # inserts before the first ACTIVATE overlaps the preamble instead of
        # serializing after the setup wait.
        actscratch = sb("actscratch", [1, 4], f32)
        nc.scalar.activation(
            out=actscratch[0:1, :], in_=actscratch[0:1, :], func=Act.Sign
        )

        # ---- PE: HAM warm-up on garbage SBUF (results never read) ----
        for _ in range(NWARM):
            nc.tensor.matmul(
                out=warm_ps[:, :], lhsT=warm_a[:, 0:128], rhs=warm_a[:, 0:NW],
                start=True, stop=True,
            )

        # ---- DVE: stacked setup over all NB blocks ----
        pzc = psb[:, 0 * NB : 1 * NB]
        pyc = psb[:, 1 * NB : 2 * NB]
        pxc = psb[:, 2 * NB : 3 * NB]
        jcc = psb[:, 3 * NB : 4 * NB]
        yfl = col[:, 0 * NB : 1 * NB]
        ypr = col[:, 1 * NB : 2 * NB]
        dyj = col[:, 2 * NB : 3 * NB]
        ccc = col[:, 3 * NB : 4 * NB]

        nc.vector.wait_ge(s_in, 16)
        MAGIC = float(12582912.0)  # 1.5 * 2**23; floor via round(py - 0.5)
        nc.vector.tensor_scalar(
            out=yfl, in0=pyc, scalar1=-0.5, scalar2=MAGIC, op0=op.add, op1=op.add
        )
        nc.vector.drain()
        nc.vector.tensor_scalar(
            out=yfl, in0=yfl, scalar1=MAGIC, scalar2=None, op0=op.subtract
        )
        nc.vector.drain()
        nc.vector.tensor_tensor(out=ypr, in0=yfl, in1=jcc, op=op.add)
        nc.vector.drain()
        nc.vector.tensor_tensor(out=dyj, in0=ypr, in1=pyc, op=op.subtract)
        nc.vector.drain()
        nc.vector.tensor_tensor(out=dyj, in0=dyj, in1=dyj, op=op.mult)
        nc.vector.drain()
        nc.vector.tensor_scalar(
            out=ccc, in0=dyj, scalar1=-1.0, scalar2=R2, op0=op.mult, op1=op.add
        )
        nc.vector.drain()
        nc.vector.wait_ge(s_st, 16)
        zv = zgb[:, :].rearrange("p z -> p () z").to_broadcast([128, NB, DLOC])
        nc.vector.tensor_tensor(
            out=dzt[:, :].rearrange("p (blk z) -> p blk z", z=DLOC),
            in0=zv,
            in1=pzc.rearrange("p blk -> p blk ()").to_broadcast([128, NB, DLOC]),
            op=op.subtract,
        )
        nc.vector.drain()
        nc.vector.tensor_tensor(out=dzt[:, :], in0=dzt[:, :], in1=dzt[:, :], op=op.mult)
        nc.vector.drain()
        nc.vector.tensor_tensor(
            out=tts[:, :].rearrange("p (blk z) -> p blk z", z=DLOC),
            in0=ccc.rearrange("p blk -> p blk ()").to_broadcast([128, NB, DLOC]),
            in1=dzt[:, :].rearrange("p (blk z) -> p blk z", z=DLOC),
            op=op.subtract,
        )
        nc.vector.wait_ge(s_st, 32)
        xv = xgb[:, :].rearrange("p x -> p () x").to_broadcast([128, NB, W])
        dxt = sb("dxt", [128, NB * W], f32)
        nc.vector.tensor_tensor(
            out=dxt[:, :].rearrange("p (blk x) -> p blk x", x=W),
            in0=xv,
            in1=pxc.rearrange("p blk -> p blk ()").to_broadcast([128, NB, W]),
            op=op.subtract,
        ).then_inc(s_dve, 1)
        c["dve"] += 1
        dxt_done = c["dve"]
        nc.vector.tensor_tensor(
            out=oh[:, :].rearrange("p (blk x) -> p blk x", x=W),
            in0=xv,
            in1=ypr.rearrange("p blk -> p blk ()").to_broadcast([128, NB, W]),
            op=op.is_equal,
        ).then_inc(s_dve, 1)
        c["dve"] += 1
        setup_done = c["dve"]

        # ACT: square dxt -> dx2 in parallel with DVE's one-hot build.
        nc.scalar.wait_ge(s_dve, dxt_done)
        nc.scalar.activation(
            out=dx2[:, :], in_=dxt[:, :], func=Act.Square
        ).then_inc(s_act, 1)
        c["act"] += 1
        act_setup = c["act"]

        # DVE indicators read ACT-produced dx2.
        nc.vector.wait_ge(s_act, act_setup)

        def blk(b, g):
            return b * NRG + g

        # Emit one group's rhs indicators on engine E ("dve" or "act").
        def emit_rhs(gi, b, zg, E):
            rt = rhs_t[gi]
            for g in range(NRG):
                rg = rgs[g]
                for zi in range(ZG):
                    z = zg * ZG + zi
                    dst = rt[:rg, zi * W : (zi + 1) * W] if NRG == 1 else None
                    # NRG>1 fallback: separate row-group tiles not supported in
                    # the raw path; kernel() guarantees NRG == 1.
                    src = dx2[:rg, blk(b, g) * W : blk(b, g) * W + W]
                    tcol = tts[:rg, blk(b, g) * DLOC + z : blk(b, g) * DLOC + z + 1]
                    if E == "dve":
                        ins = nc.vector.tensor_scalar(
                            out=dst, in0=src, scalar1=tcol, scalar2=None, op0=op.is_le
                        )
                    else:
                        ins = nc.scalar.activation(
                            out=dst, in_=src, func=Act.Relu, bias=tcol, scale=-1.0
                        )
            ins.then_inc(s_dve if E == "dve" else s_act, 1)
            c[E] += 1
            return (E, c[E])

        nc.tensor.wait_ge(s_dve, setup_done)
        groups = [(b, zg) for b in range(B) for zg in range(NZG)]
        rhs_ready = {}
        thr_done = {}
        ps1_mm = {}
        ps0_mm = {}

        # DVE stream: groups 0,2 then (interleaved below) thresholds + 4,6.
        # ACT stream: groups 1,3 then thresholds + 5,7.
        # Emission order per engine defines execution order on that engine;
        # cross-engine ordering is via the counting semaphores only.
        for gi in (0, 1, 2, 3):
            b, zg = groups[gi]
            rhs_ready[gi] = emit_rhs(gi, b, zg, "dve" if gi % 2 == 0 else "act")

        # PE: real matmuls for all 8 groups (interleave emission is irrelevant
        # for PE since it's a single engine stream; waits do the gating).
        def emit_pe(gi):
            b, zg = groups[gi]
            pi = gi // 2           # pair index 0..3
            slot = pi % 3
            co = (zg % 2) * 512
            E, v = rhs_ready[gi]
            sem = s_dve if E == "dve" else s_act
            nc.tensor.wait_ge(sem, v)
            if pi >= 3:
                TE, tv = thr_done[pi - 3]
                if TE == "both":
                    nc.tensor.wait_ge(s_dve, tv[0])
                    nc.tensor.wait_ge(s_act, tv[1])
                else:
                    nc.tensor.wait_ge(s_dve if TE == "dve" else s_act, tv)
            nc.tensor.matmul(
                out=ps0[slot][:, co : co + NW],
                lhsT=oh[: rgs[0], blk(b, 0) * W : blk(b, 0) * W + 128],
                rhs=rhs_t[gi][: rgs[0], :],
                start=True, stop=True,
            ).then_inc(s_pe, 1)
            c["pe"] += 1
            ps0_mm[gi] = c["pe"]
            nc.tensor.matmul(
                out=ps1[b][32 * zg : 32 * zg + 32, :],
                lhsT=oh[: rgs[0], blk(b, 0) * W + 128 : blk(b, 0) * W + 160],
                rhs=rhs_t[gi][: rgs[0], :],
                start=True, stop=True,
                tile_position=(0, 32 * zg),
            ).then_inc(s_pe, 1)
            c["pe"] += 1
            ps1_mm[gi] = c["pe"]

        def emit_thr_pair(pi, E):
            # threshold for ps0 pair pi (groups 2pi, 2pi+1) -> ob0[pi]
            slot = pi % 3
            need = max(ps0_mm[2 * pi], ps0_mm[2 * pi + 1])
            if E == "both":
                # split halves across DVE and ACT to halve the latency of the
                # last threshold on the critical path
                nc.vector.wait_ge(s_pe, need)
                nc.vector.tensor_scalar(
                    out=ob0[pi][:, 0:512], in0=ps0[slot][:, 0:512],
                    scalar1=0.0, scalar2=None, op0=op.is_gt,
                ).then_inc(s_dve, 1)
                c["dve"] += 1
                nc.scalar.wait_ge(s_pe, need)
                nc.scalar.activation(
                    out=ob0[pi][:, 512:1024], in_=ps0[slot][:, 512:1024],
                    func=Act.Sign,
                ).then_inc(s_act, 1)
                c["act"] += 1
                thr_done[pi] = ("both", (c["dve"], c["act"]))
                return
            if E == "dve":
                nc.vector.wait_ge(s_pe, need)
                ins = nc.vector.tensor_scalar(
                    out=ob0[pi][:, :], in0=ps0[slot][:, :], scalar1=0.0,
                    scalar2=None, op0=op.is_gt,
                )
                ins.then_inc(s_dve, 1)
                c["dve"] += 1
                thr_done[pi] = ("dve", c["dve"])
            else:
                nc.scalar.wait_ge(s_pe, need)
                ins = nc.scalar.activation(
                    out=ob0[pi][:, :], in_=ps0[slot][:, :], func=Act.Sign
                )
                ins.then_inc(s_act, 1)
                c["act"] += 1
                thr_done[pi] = ("act", c["act"])

        ps1_thr = {}

        def emit_ps1_thr(pi, E="dve"):
            # threshold the [64, NW] ps1 slice of pair pi (groups 2pi, 2pi+1)
            b = (2 * pi) // NZG
            half = (2 * pi) % NZG // 2          # 0 or 1 within the batch
            need = max(ps1_mm[2 * pi], ps1_mm[2 * pi + 1])
            if E == "dve":
                nc.vector.wait_ge(s_pe, need)
                nc.vector.tensor_scalar(
                    out=ob1[b][64 * half : 64 * half + 64, :],
                    in0=ps1[b][64 * half : 64 * half + 64, :],
                    scalar1=0.0, scalar2=None, op0=op.is_gt,
                ).then_inc(s_dve, 1)
                c["dve"] += 1
                ps1_thr[pi] = ("dve", c["dve"])
            else:
                nc.scalar.wait_ge(s_pe, need)
                nc.scalar.activation(
                    out=ob1[b][64 * half : 64 * half + 64, :],
                    in_=ps1[b][64 * half : 64 * half + 64, :],
                    func=Act.Sign,
                ).then_inc(s_act, 1)
                c["act"] += 1
                ps1_thr[pi] = ("act", c["act"])

        emit_pe(0)
        emit_pe(1)
        emit_pe(2)
        emit_pe(3)
        for gi in (4, 5, 6, 7):
            b, zg = groups[gi]
            rhs_ready[gi] = emit_rhs(gi, b, zg, "dve" if gi % 2 == 0 else "act")
        emit_pe(4)
        emit_thr_pair(0, "dve")
        emit_ps1_thr(0, "dve")
        emit_pe(5)
        emit_thr_pair(1, "act")
        emit_ps1_thr(1, "act")
        emit_pe(6)
        emit_pe(7)
        emit_thr_pair(2, "dve")
        emit_ps1_thr(2, "act")
        emit_thr_pair(3, "both")
        emit_ps1_thr(3, "act")

        # ---- SP: stores, gated on the producing threshold ----
        def store_pair(pi):
            b = (2 * pi) // NZG
            zg0 = (2 * pi) % NZG
            zl = zg0 * ZG
            TE, tv = thr_done[pi]
            if TE == "both":
                nc.sync.wait_ge(s_dve, tv[0])
                nc.sync.wait_ge(s_act, tv[1])
            else:
                nc.sync.wait_ge(s_dve if TE == "dve" else s_act, tv)
            nc.sync.dma_start(
                out=out_d[b, 0:128, zl : zl + 2 * ZG, :].rearrange(
                    "p (h z) x -> p h z x", h=2
                ),
                in_=ob0[pi][:, :].rearrange("p (h c) -> p h c", h=2)[
                    :, :, 0:NW
                ].rearrange("p h (z x) -> p h z x", z=ZG),
            ).then_inc(s_st, 16).then_inc(s_st, 16)
            c["st"] += 16

        def store_ob1(pi, ring="sync"):
            # one contiguous blocked store per zgroup-pair; host reassembles
            b = (2 * pi) // NZG
            half = (2 * pi) % NZG // 2
            eng = nc.sync if ring == "sync" else nc.scalar
            TE, tv = ps1_thr[pi]
            eng.wait_ge(s_dve if TE == "dve" else s_act, tv)
            eng.dma_start(
                out=out1_d[b, 64 * half : 64 * half + 64, :],
                in_=ob1[b][64 * half : 64 * half + 64, :],
            ).then_inc(s_st, 16)
            c["st"] += 16

        store_pair(0)
        store_ob1(0)
        store_pair(1)
        store_ob1(1)
        store_pair(2)
        store_pair(3)
        store_ob1(2, ring="scalar")
        store_ob1(3, ring="scalar")

        # ---- GPSIMD: wait for all stores to land, then reset our sems so
        # the NEFF can be executed again from a clean state. ----
        nc.gpsimd.wait_ge(s_st, c["st"])
        nc.gpsimd.dma_reset(sem_range)
        nc.gpsimd.sem_clear(sem_range)

    nc.finalize()
    return nc




def _build_program_tile(npts):
    from contextlib import ExitStack

    import concourse.mybir as mybir
    import concourse.tile as tile
    from concourse import bacc

    f32 = mybir.dt.float32
    bf16 = mybir.dt.bfloat16
    op = mybir.AluOpType

    K = npts * J                     # k-rows per batch
    NRG = (K + 127) // 128           # row groups per batch
    KP = 128 * NRG                   # padded k-rows per batch
    NB = B * NRG                     # total (b, g) blocks, stacked along free dim
    rgs = [min(128, K - 128 * g) for g in range(NRG)]  # rows per group

    nc = bacc.Bacc(trn_type="TRN2")

    # pts2[c, blk, row]: c in (pz, py, px, j); blk = b*NRG + g; rows padded
    # with far-away dummy points (never match anything).
    pts_d = nc.dram_tensor("pts", [4, NB, 128], f32, kind="ExternalInput")
    zgb_d = nc.dram_tensor("zgridb", [128, DLOC], f32, kind="ExternalInput")
    xgb_d = nc.dram_tensor("xgridb", [128, W], f32, kind="ExternalInput")
    out_d = nc.dram_tensor("out", [B, H, DLOC, W], f32, kind="ExternalOutput")

    with tile.TileContext(nc) as tc, ExitStack() as ctx:
        const = ctx.enter_context(tc.tile_pool(name="const", bufs=1))
        setup = ctx.enter_context(tc.tile_pool(name="setup", bufs=1))
        rhsp = ctx.enter_context(tc.tile_pool(name="rhsp", bufs=4))
        outp = ctx.enter_context(tc.tile_pool(name="outp", bufs=8))
        psump = ctx.enter_context(tc.tile_pool(name="psump", bufs=4, space="PSUM"))
        warmp = ctx.enter_context(tc.tile_pool(name="warmp", bufs=1, space="PSUM"))

        xgb = const.tile([128, W], f32, name="xgb")
        nc.sync.dma_start(out=xgb[:, :], in_=xgb_d[:, :])
        zgb = const.tile([128, DLOC], f32, name="zgb")
        nc.sync.dma_start(out=zgb[:, :], in_=zgb_d[:, :])
        psb = setup.tile([128, 4 * NB], f32, name="psb")
        nc.sync.dma_start(out=psb[:, :], in_=pts_d[:, :])

        # PE HAM warm-up: dummy matmuls while setup runs, so the real matmuls
        # start at 2.4 GHz (the clock gate needs ~3.4us of PE activity).
        warm_a = const.tile([128, 512], bf16, name="warm_a")
        nc.vector.memset(warm_a[:, :], 0.0)
        warm_ps = warmp.tile([128, 512], f32, name="warm_ps")
        for w in range(14):
            nc.tensor.matmul(
                out=warm_ps[:, :], lhsT=warm_a[:, 0:128], rhs=warm_a[:, :],
                start=True, stop=True,
            )

        # ---- Stacked setup over all NB blocks at once (few instructions) ----
        pzc = psb[:, 0 * NB : 1 * NB]
        pyc = psb[:, 1 * NB : 2 * NB]
        pxc = psb[:, 2 * NB : 3 * NB]
        jcc = psb[:, 3 * NB : 4 * NB]

        col = setup.tile([128, 4 * NB], f32, name="col")
        yfl = col[:, 0 * NB : 1 * NB]
        ypr = col[:, 1 * NB : 2 * NB]
        dyj = col[:, 2 * NB : 3 * NB]
        cc = col[:, 3 * NB : 4 * NB]
        # floor(py) = round(py - 0.5) via the fp32 magic-number trick (a tie at
        # integer py may give floor-1, which only shifts the 10-row window;
        # rows stay self-consistent so the result is unchanged).
        MAGIC = float(12582912.0)  # 1.5 * 2**23
        nc.vector.tensor_scalar(
            out=yfl, in0=pyc, scalar1=-0.5, scalar2=MAGIC, op0=op.add, op1=op.add
        )
        nc.vector.tensor_scalar(
            out=yfl, in0=yfl, scalar1=MAGIC, scalar2=None, op0=op.subtract
        )
        nc.vector.tensor_tensor(out=ypr, in0=yfl, in1=jcc, op=op.add)
        nc.vector.tensor_tensor(out=dyj, in0=ypr, in1=pyc, op=op.subtract)
        nc.vector.tensor_tensor(out=dyj, in0=dyj, in1=dyj, op=op.mult)
        nc.vector.tensor_scalar(
            out=cc, in0=dyj, scalar1=-1.0, scalar2=R2, op0=op.mult, op1=op.add
        )

        # t[k, (blk z)] = cc - (z - pz)^2
        dzt = setup.tile([128, NB * DLOC], f32, name="dzt")
        zgb_v = zgb[:, :].rearrange("p z -> p () z").to_broadcast([128, NB, DLOC])
        nc.vector.tensor_tensor(
            out=dzt[:, :].rearrange("p (blk z) -> p blk z", z=DLOC),
            in0=zgb_v,
            in1=pzc.rearrange("p blk -> p blk ()").to_broadcast([128, NB, DLOC]),
            op=op.subtract,
        )
        nc.vector.tensor_tensor(out=dzt[:, :], in0=dzt[:, :], in1=dzt[:, :], op=op.mult)
        tts = setup.tile([128, NB * DLOC], f32, name="tts")
        nc.vector.tensor_tensor(
            out=tts[:, :].rearrange("p (blk z) -> p blk z", z=DLOC),
            in0=cc.rearrange("p blk -> p blk ()").to_broadcast([128, NB, DLOC]),
            in1=dzt[:, :].rearrange("p (blk z) -> p blk z", z=DLOC),
            op=op.subtract,
        )

        # dx2[k, (blk x)] = (x - px)^2
        dx2 = setup.tile([128, NB * W], f32, name="dx2")
        xgb_v = xgb[:, :].rearrange("p x -> p () x").to_broadcast([128, NB, W])
        nc.vector.tensor_tensor(
            out=dx2[:, :].rearrange("p (blk x) -> p blk x", x=W),
            in0=xgb_v,
            in1=pxc.rearrange("p blk -> p blk ()").to_broadcast([128, NB, W]),
            op=op.subtract,
        )
        nc.vector.tensor_tensor(out=dx2[:, :], in0=dx2[:, :], in1=dx2[:, :], op=op.mult)

        # onehot[k, (blk y)] = (y == floor(py) + j), bf16 for the matmul
        oh = setup.tile([128, NB * W], bf16, name="oh")
        nc.vector.tensor_tensor(
            out=oh[:, :].rearrange("p (blk x) -> p blk x", x=W),
            in0=xgb_v,
            in1=ypr.rearrange("p blk -> p blk ()").to_broadcast([128, NB, W]),
            op=op.is_equal,
        )

        def blk(b, g):
            return b * NRG + g

        NW = ZG * W  # 480
        # y rows 128..159 of all 4 zgroups of a batch share one [128, NW] PSUM
        # tile (partitions 32*zg + (y-128)) -> one threshold per batch.
        ps1big = {
            b: psump.tile([128, NW], f32, name=f"ps1_{b}", tag=f"ps1_{b}", bufs=1)
            for b in range(B)
        }
        ob1big = {b: outp.tile([128, NW], f32, name=f"ob1_{b}", bufs=1) for b in range(B)}

        for b in range(B):
            for zg in range(NZG):
                rhs = []
                for g in range(NRG):
                    rg = rgs[g]
                    rhs_g = rhsp.tile([128, NW], bf16, name=f"rhs{g}", tag=f"rhs{g}")
                    for zi in range(ZG):
                        z = zg * ZG + zi
                        dst = rhs_g[:rg, zi * W : (zi + 1) * W]
                        src = dx2[:rg, blk(b, g) * W : blk(b, g) * W + W]
                        tcol = tts[:rg, blk(b, g) * DLOC + z : blk(b, g) * DLOC + z + 1]
                        if zi == 1:
                            # ScalarE: relu(t - dx2) — positive iff inside,
                            # which is all the (plane > 0) threshold needs.
                            nc.scalar.activation(
                                out=dst, in_=src,
                                func=mybir.ActivationFunctionType.Relu,
                                bias=tcol, scale=-1.0,
                            )
                        else:
                            # DVE: 0/1 indicator (dx2 <= t)
                            nc.vector.tensor_scalar(
                                out=dst, in0=src, scalar1=tcol, scalar2=None,
                                op0=op.is_le,
                            )
                    rhs.append(rhs_g)

                ps0 = psump.tile([128, NW], f32, name="ps0", tag="ps0")
                for g in range(NRG):
                    rg = rgs[g]
                    nc.tensor.matmul(
                        out=ps0[:, :],
                        lhsT=oh[:rg, blk(b, g) * W : blk(b, g) * W + 128],
                        rhs=rhs[g][:rg, :],
                        start=(g == 0), stop=(g == NRG - 1),
                    )
                for g in range(NRG):
                    rg = rgs[g]
                    nc.tensor.matmul(
                        out=ps1big[b][32 * zg : 32 * zg + 32, :],
                        lhsT=oh[:rg, blk(b, g) * W + 128 : blk(b, g) * W + 160],
                        rhs=rhs[g][:rg, :],
                        start=(g == 0), stop=(g == NRG - 1),
                        tile_position=(0, 32 * zg),
                    )

                ob0 = outp.tile([128, NW], f32, name="ob0", tag="ob0")
                if zg % 2 == 0:
                    nc.scalar.activation(
                        out=ob0[:, :], in_=ps0[:, :],
                        func=mybir.ActivationFunctionType.Sign,
                    )
                else:
                    nc.vector.tensor_scalar(
                        out=ob0[:, :], in0=ps0[:, :], scalar1=0.0, scalar2=None,
                        op0=op.is_gt,
                    )
                zl = zg * ZG
                nc.sync.dma_start(
                    out=out_d[b, 0:128, zl : zl + ZG, :],
                    in_=ob0[:, :].rearrange("p (z x) -> p z x", z=ZG),
                )

            # One threshold + 4 slice-stores for the y>=128 remainder rows.
            nc.vector.tensor_scalar(
                out=ob1big[b][:, :], in0=ps1big[b][:, :], scalar1=0.0, scalar2=None,
                op0=op.is_gt,
            )
            for zg in range(NZG):
                zl = zg * ZG
                nc.sync.dma_start(
                    out=out_d[b, 128:160, zl : zl + ZG, :],
                    in_=ob1big[b][32 * zg : 32 * zg + 32, :].rearrange(
                        "p (z x) -> p z x", z=ZG
                    ),
                )

    nc.finalize()
    return nc



def _build_in_maps(coords: np.ndarray):
    coords = np.ascontiguousarray(coords, dtype=np.float32)
    assert coords.shape == (B * P, 3)

    sel = []
    maxn = 0
    for core in range(NCORES):
        z0, z1 = core * DLOC, (core + 1) * DLOC
        per_b = []
        for b in range(B):
            cb = coords[b * P : (b + 1) * P]
            m = (cb[:, 0] > z0 - 5.001) & (cb[:, 0] < z1 + 5.001)
            pb = cb[m]
            per_b.append(pb)
            maxn = max(maxn, len(pb))
        sel.append(per_b)
    NPTS = max(12, maxn)
    if NPTS > 12:
        NPTS = ((NPTS + 3) // 4) * 4  # bucket to limit recompiles

    K = NPTS * J
    NRG = (K + 127) // 128
    NB = B * NRG
    jcol = np.tile(np.arange(-4, 6, dtype=np.float32), NPTS)

    xgridb = np.ascontiguousarray(
        np.broadcast_to(np.arange(W, dtype=np.float32), (128, W))
    )

    in_maps = []
    for core in range(NCORES):
        zbase = core * DLOC
        zgridb = np.ascontiguousarray(
            np.broadcast_to(
                np.arange(zbase, zbase + DLOC, dtype=np.float32), (128, DLOC)
            )
        )
        pts2 = np.empty((4, NB, 128), dtype=np.float32)
        pts2[0:3] = 1.0e9
        pts2[3] = 0.0
        for b in range(B):
            pb = sel[core][b]
            krows = np.repeat(pb, J, axis=0)
            n = krows.shape[0]
            for g in range(NRG):
                lo, hi = g * 128, min((g + 1) * 128, n)
                if lo >= n:
                    break
                bk = b * NRG + g
                pts2[0, bk, 0 : hi - lo] = krows[lo:hi, 0]
                pts2[1, bk, 0 : hi - lo] = krows[lo:hi, 1]
                pts2[2, bk, 0 : hi - lo] = krows[lo:hi, 2]
                pts2[3, bk, 0 : hi - lo] = jcol[lo:hi]
        in_maps.append(
            {
                # [128, 4*NB]: the SBUF layout, so the load is contiguous
                "pts": np.ascontiguousarray(
                    pts2.reshape(4 * NB, 128).T
                ),
                "zgridb": zgridb,
                "xgridb": xgridb,
            }
        )
    return NPTS, in_maps



def _get_program(npts):
    if npts not in _prog_cache:
        if npts * J <= 128:
            _prog_cache[npts] = _build_program_fast(npts)
        else:
            _prog_cache[npts] = _build_program_tile(npts)
    return _prog_cache[npts]


def kernel(x: np.ndarray, coords: np.ndarray) -> np.ndarray:
    from concourse.bass_utils import run_bass_kernel_spmd

    assert x.shape == (B, 4, D, H, W)
    NPTS, in_maps = _build_in_maps(coords)
    nc = _get_program(NPTS)
    res = run_bass_kernel_spmd(nc, in_maps, list(range(NCORES)))

    full = np.empty((B, 1, D, H, W), dtype=np.float32)
    for core in range(NCORES):
        o = res.results[core]["out"]  # [B, H, DLOC, W]
        zsl = slice(core * DLOC, (core + 1) * DLOC)
        full[:, 0, zsl] = o.transpose(0, 2, 1, 3)
        if "out1" in res.results[core]:
            # fast path: y>=128 rows arrive blocked as [b, (zg, yo), (zi, x)]
            o1 = res.results[core]["out1"].reshape(B, NZG, 32, ZG, W)
            full[:, 0, zsl, 128:160, :] = o1.transpose(0, 1, 3, 2, 4).reshape(
                B, DLOC, 32, W
            )
    return full



# revision 8
# speedup vs baseline: 1.2765x; 1.2765x over previous
"""DistMaps kernel for Trainium2 (Bass), SPMD over 8 NeuronCores.

Problem: out[b, 0, z, y, x] = 1.0 if min_p ((z-pz)^2 + (y-py)^2 + (x-px)^2) <= 25
over the 24 points p of batch b, else 0.0.  (The x input is only used for its
shape.)

Strategy (v2)
-------------
Shard the volume over D (96 = 8 cores x 12 z-slices); each core gets only the
points whose +-5 z-window intersects its slab (<= 12, padded with far-away
dummies), so no collective is needed.

Per z-slice the plane is a union of disks; row by row a disk is an x-interval:
ind[k, x] = (x-px)^2 <= 25 - (y-py)^2 - (z-pz)^2 for k-rows (point, row offset
j in -4..5).  With onehot[k, y] = (y == floor(py)+j):

  plane[y, x] = sum_k onehot[k, y] * ind_z[k, x]   (TensorE matmul, bf16)
  out = plane > 0                                  (DVE is_gt / ACT Sign -> u8)

v2 changes vs v1 (28.9us):
  * all per-point tables (dx2[k,x], tts[k,z], onehot[k,y]) precomputed on the
    HOST and shipped as two small DMAs (~215 KB) -- removes the 4us serialized
    DVE setup chain from the critical path.
  * rhs indicators built in 4 fused DVE tensor_tensor ops (double-broadcast)
    instead of 24 sliced ops.
  * 6 matmuls instead of 16: per batch one 960-col matmul per zgroup-pair for
    y<128 and ONE 1920-col matmul for the y>=128 remainder (both batches share
    one [64,1920] PSUM via tile_position).
  * thresholds write uint8 (0/1 is exact); the host casts to f32 on unshard.
    4x fewer output bytes -> store drain ~2us instead of ~8us.
  * stores chase each pair threshold on the sync ring (overlapped with the
    rest of the pipeline).
"""

import numpy as np

B = 2
D, H, W = 96, 160, 160
P = 24
J = 10
NCORES = 8
DLOC = D // NCORES   # 12
ZG = 3               # z-slices per matmul group (tile fallback)
NZG = DLOC // ZG     # 4
R2 = 25.0
NPTS_FAST = 12
K = NPTS_FAST * J    # 120 k-rows per batch (fast path)
ZH = DLOC // 2       # 6 z-slices per zgroup-half (fast path)
NWARM = 4
WARMC = 480

_prog_cache = {}


def _build_program_fast(npts):
    import concourse.mybir as mybir
    from concourse import bacc

    assert npts == NPTS_FAST
    f32 = mybir.dt.float32
    bf16 = mybir.dt.bfloat16
    u8 = mybir.dt.uint8
    op = mybir.AluOpType
    Act = mybir.ActivationFunctionType

    NW = ZH * W            # 960 cols per zgroup-half
    NWD = DLOC * W         # 1920 cols per batch

    nc = bacc.Bacc(trn_type="TRN2")

    # tab: [128, 2*W + 2*DLOC] f32 = dx2[k, (b,x)] ++ tts[k, (b,z)]
    tab_d = nc.dram_tensor("tab", [128, 2 * W + 2 * DLOC], f32, kind="ExternalInput")
    # oh: [128, 2*H] bf16 = onehot[k, (b,y)]
    oh_d = nc.dram_tensor("oh", [128, 2 * H], bf16, kind="ExternalInput")
    # out0: pair stores, pair p = b*2 + h covers z in [h*6, h*6+6), y in [0,128)
    out0_d = nc.dram_tensor("out0", [2 * B, 128, NW], u8, kind="ExternalOutput")
    # out1: y in [128,160): partitions (b,yo), cols (z,x)
    out1_d = nc.dram_tensor("out1", [2 * 32, NWD], u8, kind="ExternalOutput")

    s_in = nc.alloc_semaphore("s_in")
    s_dve = nc.alloc_semaphore("s_dve")
    s_act = nc.alloc_semaphore("s_act")
    s_pe = nc.alloc_semaphore("s_pe")
    s_st = nc.alloc_semaphore("s_st")
    sem_nums = [s.num for s in (s_in, s_dve, s_act, s_pe, s_st)]
    assert max(sem_nums) - min(sem_nums) + 1 == len(sem_nums), sem_nums
    sem_range = range(min(sem_nums), max(sem_nums) + 1)

    tab = nc.alloc_sbuf_tensor("tab_s", [128, 2 * W + 2 * DLOC], f32)
    oh = nc.alloc_sbuf_tensor("oh_s", [128, 2 * H], bf16)
    warm_a = nc.alloc_sbuf_tensor("warm_a", [128, 512], bf16)
    rhs = [nc.alloc_sbuf_tensor(f"rhs{b}", [128, NWD], bf16) for b in range(B)]
    ob0 = [nc.alloc_sbuf_tensor(f"ob0_{p}", [128, NW], u8) for p in range(2 * B)]
    ob1 = nc.alloc_sbuf_tensor("ob1", [64, NWD], u8)

    # Matmul dsts must sit inside a single 512-f32 PSUM bank: 480-col chunks
    # are placed at bank-aligned offsets (0, 512, ...), thresholds read the
    # banks with a strided AP and pack the result contiguously.
    ps0 = [nc.alloc_psum_tensor(f"ps0_{i}", [128, 1024], f32) for i in range(2)]
    ps1 = nc.alloc_psum_tensor("ps1", [64, 2048], f32)

    # ---- SP: input table load (the DVE-critical one) ----
    nc.sync.dma_start(out=tab[:, :], in_=tab_d[:, :], single_packet=True).then_inc(
        s_in, 16
    )

    # ---- ACT ring: onehot load (PE-critical), then dummy activation so the
    # ~1.3us ACT_TABLE_LOAD walrus inserts before the first ACTIVATE runs
    # during the input-DMA flight instead of before the first threshold. ----
    nc.scalar.dma_start(out=oh[:, :], in_=oh_d[:, :], single_packet=True).then_inc(
        s_st, 16
    )
    actscratch = nc.alloc_sbuf_tensor("actscratch", [1, 4], f32)
    nc.scalar.activation(out=actscratch[0:1, :], in_=actscratch[0:1, :], func=Act.Sign)

    # ---- PE: HAM warm-up on garbage SBUF (results never read) ----
    for _ in range(NWARM):
        nc.tensor.matmul(
            out=ps0[0][:, 0:WARMC], lhsT=warm_a[:, 0:128], rhs=warm_a[:, 0:WARMC],
            start=True, stop=True,
        )

    # ---- DVE: rhs indicator builds, one fused op per (b, half) ----
    # rhs[b][k, (z, x)] = (dx2[k, x] <= tts[k, z]) as bf16 0/1
    dx2 = [tab[:K, b * W : (b + 1) * W] for b in range(B)]
    tts = [tab[:K, 2 * W + b * DLOC : 2 * W + (b + 1) * DLOC] for b in range(B)]
    c = {"dve": 0, "act": 0, "pe": 0, "st": 0}
    rhs_done = {}
    nc.vector.wait_ge(s_in, 16)
    for b in range(B):
        for h in range(2):
            nc.vector.tensor_tensor(
                out=rhs[b][:K, h * NW : (h + 1) * NW].rearrange(
                    "p (z x) -> p z x", x=W
                ),
                in0=dx2[b].rearrange("p x -> p () x").to_broadcast([K, ZH, W]),
                in1=tts[b][:, h * ZH : (h + 1) * ZH]
                .rearrange("p z -> p z ()")
                .to_broadcast([K, ZH, W]),
                op=op.is_le,
            ).then_inc(s_dve, 1)
            c["dve"] += 1
            rhs_done[(b, h)] = c["dve"]

    # ---- PE: 6 real matmuls ----
    oh128 = [oh[:K, b * H : b * H + 128] for b in range(B)]
    oh32 = [oh[:K, b * H + 128 : b * H + 160] for b in range(B)]
    mm_done = {}
    thr_done = {}
    nc.tensor.wait_ge(s_st, 16)   # oh landed

    HW2 = NW // 2  # 480 — the ISA caps matmul moving size at 512 elements

    def emit_mm_pair(b, h):
        # ps0 slot h%2... cycle A,B,A,B across (b,h) pairs
        pi = b * 2 + h
        slot = pi % 2
        nc.tensor.wait_ge(s_dve, rhs_done[(b, h)])
        if pi >= 2:
            # slot reuse: wait for the threshold of pair pi-2 to have drained
            eng, v = thr_done[pi - 2]
            nc.tensor.wait_ge(s_dve if eng == "dve" else s_act, v)
        for q in range(2):
            ins = nc.tensor.matmul(
                out=ps0[slot][:, q * 512 : q * 512 + HW2],
                lhsT=oh128[b],
                rhs=rhs[b][:K, h * NW + q * HW2 : h * NW + (q + 1) * HW2],
                start=True, stop=True,
            )
        ins.then_inc(s_pe, 1)
        c["pe"] += 1
        mm_done[pi] = c["pe"]

    def emit_mm_ps1(b):
        nc.tensor.wait_ge(s_dve, rhs_done[(b, 1)])
        for q in range(4):
            ins = nc.tensor.matmul(
                out=ps1[32 * b : 32 * b + 32, q * 512 : q * 512 + HW2],
                lhsT=oh32[b],
                rhs=rhs[b][:K, q * HW2 : (q + 1) * HW2],
                start=True, stop=True,
                tile_position=(0, 32 * b),
            )
        ins.then_inc(s_pe, 1)
        c["pe"] += 1
        mm_done[("ps1", b)] = c["pe"]

    def emit_thr(pi, E):
        slot = pi % 2
        # strided read of the two bank-aligned 480-col chunks, packed out
        src = ps0[slot][:, :].rearrange("p (q c) -> p q c", c=512)[:, :, 0:HW2]
        dst = ob0[pi][:, :].rearrange("p (q c) -> p q c", c=HW2)
        if E == "dve":
            nc.vector.wait_ge(s_pe, mm_done[pi])
            nc.vector.tensor_scalar(
                out=dst, in0=src,
                scalar1=0.0, scalar2=None, op0=op.is_gt,
            ).then_inc(s_dve, 1)
            c["dve"] += 1
            thr_done[pi] = ("dve", c["dve"])
        else:
            nc.scalar.wait_ge(s_pe, mm_done[pi])
            nc.scalar.activation(
                out=dst, in_=src, func=Act.Sign
            ).then_inc(s_act, 1)
            c["act"] += 1
            thr_done[pi] = ("act", c["act"])

    emit_mm_pair(0, 0)   # pi 0 -> slot A
    emit_mm_pair(0, 1)   # pi 1 -> slot B
    emit_mm_ps1(0)
    emit_thr(0, "dve")
    emit_thr(1, "act")
    emit_mm_pair(1, 0)   # pi 2 -> slot A (waits thr0)
    emit_mm_pair(1, 1)   # pi 3 -> slot B (waits thr1)
    emit_mm_ps1(1)
    emit_thr(2, "dve")
    emit_thr(3, "act")
    # ps1 threshold: one op for both batches, strided over the 4 bank chunks
    nc.vector.wait_ge(s_pe, mm_done[("ps1", 1)])
    nc.vector.tensor_scalar(
        out=ob1[:, :].rearrange("p (q c) -> p q c", c=HW2),
        in0=ps1[:, :].rearrange("p (q c) -> p q c", c=512)[:, :, 0:HW2],
        scalar1=0.0, scalar2=None, op0=op.is_gt,
    ).then_inc(s_dve, 1)
    c["dve"] += 1
    thr_ps1 = c["dve"]

    # ---- SP: stores chase thresholds ----
    for pi in range(4):
        eng, v = thr_done[pi]
        nc.sync.wait_ge(s_dve if eng == "dve" else s_act, v)
        nc.sync.dma_start(out=out0_d[pi, :, :], in_=ob0[pi][:, :]).then_inc(s_st, 16)
        c["st"] += 16
    nc.sync.wait_ge(s_dve, thr_ps1)
    nc.sync.dma_start(out=out1_d[:, :], in_=ob1[:, :]).then_inc(s_st, 16)
    c["st"] += 16

    # ---- GPSIMD: wait for stores + oh-load, reset sems for re-execution ----
    nc.gpsimd.wait_ge(s_st, c["st"] + 16)
    nc.gpsimd.wait_ge(s_in, 16)
    nc.gpsimd.dma_reset(sem_range)
    nc.gpsimd.sem_clear(sem_range)

    nc.finalize()
    return nc


def _build_in_maps_fast(coords: np.ndarray, sel):
    """Host-precomputed tables per core: tab = dx2 ++ tts (f32), oh (bf16)."""
    import ml_dtypes

    jcol = np.arange(-4, 6, dtype=np.float32)
    xs = np.arange(W, dtype=np.float32)
    ys = np.arange(H, dtype=np.float32)

    in_maps = []
    for core in range(NCORES):
        zbase = core * DLOC
        zs = np.arange(zbase, zbase + DLOC, dtype=np.float32)
        tab = np.zeros((128, 2 * W + 2 * DLOC), dtype=np.float32)
        oh = np.zeros((128, 2 * H), dtype=np.float32)
        for b in range(B):
            pb = sel[core][b]
            pts = np.full((NPTS_FAST, 3), 1.0e9, dtype=np.float32)
            pts[: len(pb)] = pb
            # k-rows: (point, j)
            pz = np.repeat(pts[:, 0], J)              # (K,)
            py = np.repeat(pts[:, 1], J)
            px = np.repeat(pts[:, 2], J)
            jj = np.tile(jcol, NPTS_FAST)
            yrow = np.floor(py).astype(np.float32) + jj   # (K,)
            dyj = (yrow - py).astype(np.float32) ** 2
            cc = (np.float32(R2) - dyj).astype(np.float32)
            # tts[k, z] = cc - (z - pz)^2
            dz2 = ((zs[None, :] - pz[:, None]).astype(np.float32) ** 2).astype(
                np.float32
            )
            tts = (cc[:, None] - dz2).astype(np.float32)
            dx2 = ((xs[None, :] - px[:, None]).astype(np.float32) ** 2).astype(
                np.float32
            )
            tab[:K, b * W : (b + 1) * W] = dx2
            tab[:K, 2 * W + b * DLOC : 2 * W + (b + 1) * DLOC] = tts
            oh[:K, b * H : (b + 1) * H] = (ys[None, :] == yrow[:, None]).astype(
                np.float32
            )
        in_maps.append(
            {
                "tab": np.ascontiguousarray(tab),
                "oh": np.ascontiguousarray(oh.astype(ml_dtypes.bfloat16)),
            }
        )
    return in_maps


def _select_points(coords: np.ndarray):
    coords = np.ascontiguousarray(coords, dtype=np.float32)
    assert coords.shape == (B * P, 3)
    sel = []
    maxn = 0
    for core in range(NCORES):
        z0, z1 = core * DLOC, (core + 1) * DLOC
        per_b = []
        for b in range(B):
            cb = coords[b * P : (b + 1) * P]
            m = (cb[:, 0] > z0 - 5.001) & (cb[:, 0] < z1 + 5.001)
            pb = cb[m]
            per_b.append(pb)
            maxn = max(maxn, len(pb))
        sel.append(per_b)
    return sel, maxn


# ---------------------------------------------------------------------------
# Tile-scheduled fallback for NPTS > 12 (unchanged from v1).
# ---------------------------------------------------------------------------

def _build_program_tile(npts):
    from contextlib import ExitStack

    import concourse.mybir as mybir
    import concourse.tile as tile
    from concourse import bacc

    f32 = mybir.dt.float32
    bf16 = mybir.dt.bfloat16
    op = mybir.AluOpType

    K = npts * J                     # k-rows per batch
    NRG = (K + 127) // 128           # row groups per batch
    NB = B * NRG                     # total (b, g) blocks, stacked along free dim
    rgs = [min(128, K - 128 * g) for g in range(NRG)]  # rows per group

    nc = bacc.Bacc(trn_type="TRN2")

    pts_d = nc.dram_tensor("pts", [4, NB, 128], f32, kind="ExternalInput")
    zgb_d = nc.dram_tensor("zgridb", [128, DLOC], f32, kind="ExternalInput")
    xgb_d = nc.dram_tensor("xgridb", [128, W], f32, kind="ExternalInput")
    out_d = nc.dram_tensor("out", [B, H, DLOC, W], f32, kind="ExternalOutput")

    with tile.TileContext(nc) as tc, ExitStack() as ctx:
        const = ctx.enter_context(tc.tile_pool(name="const", bufs=1))
        setup = ctx.enter_context(tc.tile_pool(name="setup", bufs=1))
        rhsp = ctx.enter_context(tc.tile_pool(name="rhsp", bufs=4))
        outp = ctx.enter_context(tc.tile_pool(name="outp", bufs=8))
        psump = ctx.enter_context(tc.tile_pool(name="psump", bufs=4, space="PSUM"))
        warmp = ctx.enter_context(tc.tile_pool(name="warmp", bufs=1, space="PSUM"))

        xgb = const.tile([128, W], f32, name="xgb")
        nc.sync.dma_start(out=xgb[:, :], in_=xgb_d[:, :])
        zgb = const.tile([128, DLOC], f32, name="zgb")
        nc.sync.dma_start(out=zgb[:, :], in_=zgb_d[:, :])
        psb = setup.tile([128, 4 * NB], f32, name="psb")
        nc.sync.dma_start(out=psb[:, :], in_=pts_d[:, :])

        warm_a = const.tile([128, 512], bf16, name="warm_a")
        nc.vector.memset(warm_a[:, :], 0.0)
        warm_ps = warmp.tile([128, 512], f32, name="warm_ps")
        for w in range(14):
            nc.tensor.matmul(
                out=warm_ps[:, :], lhsT=warm_a[:, 0:128], rhs=warm_a[:, :],
                start=True, stop=True,
            )

        pzc = psb[:, 0 * NB : 1 * NB]
        pyc = psb[:, 1 * NB : 2 * NB]
        pxc = psb[:, 2 * NB : 3 * NB]
        jcc = psb[:, 3 * NB : 4 * NB]

        col = setup.tile([128, 4 * NB], f32, name="col")
        yfl = col[:, 0 * NB : 1 * NB]
        ypr = col[:, 1 * NB : 2 * NB]
        dyj = col[:, 2 * NB : 3 * NB]
        cc = col[:, 3 * NB : 4 * NB]
        MAGIC = float(12582912.0)  # 1.5 * 2**23
        nc.vector.tensor_scalar(
            out=yfl, in0=pyc, scalar1=-0.5, scalar2=MAGIC, op0=op.add, op1=op.add
        )
        nc.vector.tensor_scalar(
            out=yfl, in0=yfl, scalar1=MAGIC, scalar2=None, op0=op.subtract
        )
        nc.vector.tensor_tensor(out=ypr, in0=yfl, in1=jcc, op=op.add)
        nc.vector.tensor_tensor(out=dyj, in0=ypr, in1=pyc, op=op.subtract)
        nc.vector.tensor_tensor(out=dyj, in0=dyj, in1=dyj, op=op.mult)
        nc.vector.tensor_scalar(
            out=cc, in0=dyj, scalar1=-1.0, scalar2=R2, op0=op.mult, op1=op.add
        )

        dzt = setup.tile([128, NB * DLOC], f32, name="dzt")
        zgb_v = zgb[:, :].rearrange("p z -> p () z").to_broadcast([128, NB, DLOC])
        nc.vector.tensor_tensor(
            out=dzt[:, :].rearrange("p (blk z) -> p blk z", z=DLOC),
            in0=zgb_v,
            in1=pzc.rearrange("p blk -> p blk ()").to_broadcast([128, NB, DLOC]),
            op=op.subtract,
        )
        nc.vector.tensor_tensor(out=dzt[:, :], in0=dzt[:, :], in1=dzt[:, :], op=op.mult)
        tts = setup.tile([128, NB * DLOC], f32, name="tts")
        nc.vector.tensor_tensor(
            out=tts[:, :].rearrange("p (blk z) -> p blk z", z=DLOC),
            in0=cc.rearrange("p blk -> p blk ()").to_broadcast([128, NB, DLOC]),
            in1=dzt[:, :].rearrange("p (blk z) -> p blk z", z=DLOC),
            op=op.subtract,
        )

        dx2 = setup.tile([128, NB * W], f32, name="dx2")
        xgb_v = xgb[:, :].rearrange("p x -> p () x").to_broadcast([128, NB, W])
        nc.vector.tensor_tensor(
            out=dx2[:, :].rearrange("p (blk x) -> p blk x", x=W),
            in0=xgb_v,
            in1=pxc.rearrange("p blk -> p blk ()").to_broadcast([128, NB, W]),
            op=op.subtract,
        )
        nc.vector.tensor_tensor(out=dx2[:, :], in0=dx2[:, :], in1=dx2[:, :], op=op.mult)

        oh = setup.tile([128, NB * W], bf16, name="oh")
        nc.vector.tensor_tensor(
            out=oh[:, :].rearrange("p (blk x) -> p blk x", x=W),
            in0=xgb_v,
            in1=ypr.rearrange("p blk -> p blk ()").to_broadcast([128, NB, W]),
            op=op.is_equal,
        )

        def blk(b, g):
            return b * NRG + g

        NW = ZG * W  # 480
        ps1big = {
            b: psump.tile([128, NW], f32, name=f"ps1_{b}", tag=f"ps1_{b}", bufs=1)
            for b in range(B)
        }
        ob1big = {b: outp.tile([128, NW], f32, name=f"ob1_{b}", bufs=1) for b in range(B)}

        for b in range(B):
            for zg in range(NZG):
                rhs = []
                for g in range(NRG):
                    rg = rgs[g]
                    rhs_g = rhsp.tile([128, NW], bf16, name=f"rhs{g}", tag=f"rhs{g}")
                    for zi in range(ZG):
                        z = zg * ZG + zi
                        dst = rhs_g[:rg, zi * W : (zi + 1) * W]
                        src = dx2[:rg, blk(b, g) * W : blk(b, g) * W + W]
                        tcol = tts[:rg, blk(b, g) * DLOC + z : blk(b, g) * DLOC + z + 1]
                        if zi == 1:
                            nc.scalar.activation(
                                out=dst, in_=src,
                                func=mybir.ActivationFunctionType.Relu,
                                bias=tcol, scale=-1.0,
                            )
                        else:
                            nc.vector.tensor_scalar(
                                out=dst, in0=src, scalar1=tcol, scalar2=None,
                                op0=op.is_le,
                            )
                    rhs.append(rhs_g)

                ps0 = psump.tile([128, NW], f32, name="ps0", tag="ps0")
                for g in range(NRG):
                    rg = rgs[g]
                    nc.tensor.matmul(
                        out=ps0[:, :],
                        lhsT=oh[:rg, blk(b, g) * W : blk(b, g) * W + 128],
                        rhs=rhs[g][:rg, :],
                        start=(g == 0), stop=(g == NRG - 1),
                    )
                for g in range(NRG):
                    rg = rgs[g]
                    nc.tensor.matmul(
                        out=ps1big[b][32 * zg : 32 * zg + 32, :],
                        lhsT=oh[:rg, blk(b, g) * W + 128 : blk(b, g) * W + 160],
                        rhs=rhs[g][:rg, :],
                        start=(g == 0), stop=(g == NRG - 1),
                        tile_position=(0, 32 * zg),
                    )

                ob0 = outp.tile([128, NW], f32, name="ob0", tag="ob0")
                if zg % 2 == 0:
                    nc.scalar.activation(
                        out=ob0[:, :], in_=ps0[:, :],
                        func=mybir.ActivationFunctionType.Sign,
                    )
                else:
                    nc.vector.tensor_scalar(
                        out=ob0[:, :], in0=ps0[:, :], scalar1=0.0, scalar2=None,
                        op0=op.is_gt,
                    )
                zl = zg * ZG
                nc.sync.dma_start(
                    out=out_d[b, 0:128, zl : zl + ZG, :],
                    in_=ob0[:, :].rearrange("p (z x) -> p z x", z=ZG),
                )

            nc.vector.tensor_scalar(
                out=ob1big[b][:, :], in0=ps1big[b][:, :], scalar1=0.0, scalar2=None,
                op0=op.is_gt,
            )
            for zg in range(NZG):
                zl = zg * ZG
                nc.sync.dma_start(
                    out=out_d[b, 128:160, zl : zl + ZG, :],
                    in_=ob1big[b][32 * zg : 32 * zg + 32, :].rearrange(
                        "p (z x) -> p z x", z=ZG
                    ),
                )

    nc.finalize()
    return nc


def _build_in_maps_tile(coords: np.ndarray, sel, NPTS):
    K = NPTS * J
    NRG = (K + 127) // 128
    NB = B * NRG
    jcol = np.tile(np.arange(-4, 6, dtype=np.float32), NPTS)

    xgridb = np.ascontiguousarray(
        np.broadcast_to(np.arange(W, dtype=np.float32), (128, W))
    )

    in_maps = []
    for core in range(NCORES):
        zbase = core * DLOC
        zgridb = np.ascontiguousarray(
            np.broadcast_to(
                np.arange(zbase, zbase + DLOC, dtype=np.float32), (128, DLOC)
            )
        )
        pts2 = np.empty((4, NB, 128), dtype=np.float32)
        pts2[0:3] = 1.0e9
        pts2[3] = 0.0
        for b in range(B):
            pb = sel[core][b]
            krows = np.repeat(pb, J, axis=0)
            n = krows.shape[0]
            for g in range(NRG):
                lo, hi = g * 128, min((g + 1) * 128, n)
                if lo >= n:
                    break
                bk = b * NRG + g
                pts2[0, bk, 0 : hi - lo] = krows[lo:hi, 0]
                pts2[1, bk, 0 : hi - lo] = krows[lo:hi, 1]
                pts2[2, bk, 0 : hi - lo] = krows[lo:hi, 2]
                pts2[3, bk, 0 : hi - lo] = jcol[lo:hi]
        in_maps.append(
            {
                "pts": np.ascontiguousarray(pts2),
                "zgridb": zgridb,
                "xgridb": xgridb,
            }
        )
    return in_maps


def _build_in_maps(coords: np.ndarray):
    sel, maxn = _select_points(coords)
    if maxn <= NPTS_FAST:
        return NPTS_FAST, _build_in_maps_fast(coords, sel)
    NPTS = ((maxn + 3) // 4) * 4
    return NPTS, _build_in_maps_tile(coords, sel, NPTS)


def _get_program(npts):
    if npts not in _prog_cache:
        if npts <= NPTS_FAST:
            _prog_cache[npts] = _build_program_fast(npts)
        else:
            _prog_cache[npts] = _build_program_tile(npts)
    return _prog_cache[npts]


def kernel(x: np.ndarray, coords: np.ndarray) -> np.ndarray:
    from concourse.bass_utils import run_bass_kernel_spmd

    assert x.shape == (B, 4, D, H, W)
    NPTS, in_maps = _build_in_maps(coords)
    nc = _get_program(NPTS)
    res = run_bass_kernel_spmd(nc, in_maps, list(range(NCORES)))

    full = np.empty((B, 1, D, H, W), dtype=np.float32)
    for core in range(NCORES):
        zsl = slice(core * DLOC, (core + 1) * DLOC)
        r = res.results[core]
        if NPTS <= NPTS_FAST:
            # out0: [4, 128, 960] u8, pair p = b*2+h -> z in [6h,6h+6), y<128
            o0 = r["out0"].reshape(B, 2, 128, ZH, W)
            full[:, 0, zsl, 0:128, :] = (
                o0.transpose(0, 1, 3, 2, 4).reshape(B, DLOC, 128, W)
            )
            # out1: [64, 1920] u8, partitions (b,yo), cols (z,x)
            o1 = r["out1"].reshape(B, 32, DLOC, W)
            full[:, 0, zsl, 128:160, :] = o1.transpose(0, 2, 1, 3)
        else:
            o = r["out"]  # [B, H, DLOC, W] f32
            full[:, 0, zsl] = o.transpose(0, 2, 1, 3)
    return full


# revision 11
# speedup vs baseline: 1.4772x; 1.1573x over previous
"""DistMaps kernel for Trainium2 (Bass), SPMD over 8 NeuronCores.

Problem: out[b, 0, z, y, x] = 1.0 if min_p ((z-pz)^2 + (y-py)^2 + (x-px)^2) <= 25
over the 24 points p of batch b, else 0.0.  (The x input is only used for its
shape.)

Strategy (v2)
-------------
Shard the volume over D (96 = 8 cores x 12 z-slices); each core gets only the
points whose +-5 z-window intersects its slab (<= 12, padded with far-away
dummies), so no collective is needed.

Per z-slice the plane is a union of disks; row by row a disk is an x-interval:
ind[k, x] = (x-px)^2 <= 25 - (y-py)^2 - (z-pz)^2 for k-rows (point, row offset
j in -4..5).  With onehot[k, y] = (y == floor(py)+j):

  plane[y, x] = sum_k onehot[k, y] * ind_z[k, x]   (TensorE matmul, bf16)
  out = plane > 0                                  (DVE is_gt / ACT Sign -> u8)

v2 changes vs v1 (28.9us):
  * all per-point tables (dx2[k,x], tts[k,z], onehot[k,y]) precomputed on the
    HOST and shipped as two small DMAs (~215 KB) -- removes the 4us serialized
    DVE setup chain from the critical path.
  * rhs indicators built in 4 fused DVE tensor_tensor ops (double-broadcast)
    instead of 24 sliced ops.
  * 6 matmuls instead of 16: per batch one 960-col matmul per zgroup-pair for
    y<128 and ONE 1920-col matmul for the y>=128 remainder (both batches share
    one [64,1920] PSUM via tile_position).
  * thresholds write uint8 (0/1 is exact); the host casts to f32 on unshard.
    4x fewer output bytes -> store drain ~2us instead of ~8us.
  * stores chase each pair threshold on the sync ring (overlapped with the
    rest of the pipeline).
"""

import numpy as np

B = 2
D, H, W = 96, 160, 160
P = 24
J = 10
NCORES = 8
DLOC = D // NCORES   # 12
ZG = 3               # z-slices per matmul group (tile fallback)
NZG = DLOC // ZG     # 4
R2 = 25.0
NPTS_FAST = 12
K = NPTS_FAST * J    # 120 k-rows per batch (fast path)
ZH = DLOC // 2       # 6 z-slices per zgroup-half (fast path)
NWARM = 10
WARMC = 480

_prog_cache = {}


def _build_program_fast(npts):
    import concourse.mybir as mybir
    from concourse import bacc

    assert npts == NPTS_FAST
    f32 = mybir.dt.float32
    bf16 = mybir.dt.bfloat16
    u8 = mybir.dt.uint8
    op = mybir.AluOpType
    Act = mybir.ActivationFunctionType

    NW = ZH * W            # 960 cols per zgroup-half
    NWD = DLOC * W         # 1920 cols per batch
    HW2 = NW // 2          # 480: matmul dsts must fit in one 512-f32 PSUM bank

    nc = bacc.Bacc(trn_type="TRN2")

    tab_d = nc.dram_tensor("tab", [128, 2 * W + 2 * DLOC], f32, kind="ExternalInput")
    oh_d = nc.dram_tensor("oh", [128, 2 * H], bf16, kind="ExternalInput")
    # out0: pair stores, pair p = b*2 + h covers z in [h*6, h*6+6), y in [0,128)
    out0_d = nc.dram_tensor("out0", [2 * B, 128, NW], u8, kind="ExternalOutput")
    # out1: y in [128,160): partitions (b, h, yo), cols (zr, x), z = h*6+zr
    out1_d = nc.dram_tensor("out1", [128, NW], u8, kind="ExternalOutput")

    s_in = nc.alloc_semaphore("s_in")
    s_dve = nc.alloc_semaphore("s_dve")
    s_act = nc.alloc_semaphore("s_act")
    s_pe = nc.alloc_semaphore("s_pe")
    s_st = nc.alloc_semaphore("s_st")
    sem_nums = [s.num for s in (s_in, s_dve, s_act, s_pe, s_st)]
    assert max(sem_nums) - min(sem_nums) + 1 == len(sem_nums), sem_nums
    sem_range = range(min(sem_nums), max(sem_nums) + 1)

    tab = nc.alloc_sbuf_tensor("tab_s", [128, 2 * W + 2 * DLOC], f32)
    oh = nc.alloc_sbuf_tensor("oh_s", [128, 2 * H], bf16)
    warm_a = nc.alloc_sbuf_tensor("warm_a", [128, 512], bf16)
    rhs = [nc.alloc_sbuf_tensor(f"rhs{b}", [128, NWD], bf16) for b in range(B)]
    ob0 = [nc.alloc_sbuf_tensor(f"ob0_{p}", [128, NW], u8) for p in range(2 * B)]
    ob1 = nc.alloc_sbuf_tensor("ob1", [128, NW], u8)

    # 3 ps0 slots (2 banks each) + ps1 (2 banks) = 8 banks
    ps0 = [nc.alloc_psum_tensor(f"ps0_{i}", [128, 1024], f32) for i in range(3)]
    ps1 = nc.alloc_psum_tensor("ps1", [128, 1024], f32)

    # ---- SP: input table load (the DVE-critical one) ----
    nc.sync.dma_start(out=tab[:, :], in_=tab_d[:, :], single_packet=True).then_inc(
        s_in, 16
    )

    # ---- ACT ring: onehot load (PE-critical), then dummy activation so the
    # ~1.3us ACT_TABLE_LOAD runs during the input-DMA flight. ----
    nc.scalar.dma_start(out=oh[:, :], in_=oh_d[:, :], single_packet=True).then_inc(
        s_st, 16
    )
    actscratch = nc.alloc_sbuf_tensor("actscratch", [1, 4], f32)
    nc.scalar.activation(out=actscratch[0:1, :], in_=actscratch[0:1, :], func=Act.Sign)

    # ---- PE: continuous HAM warm-up until the first rhs lands ----
    for _ in range(NWARM):
        nc.tensor.matmul(
            out=ps0[0][:, 0:WARMC], lhsT=warm_a[:, 0:128], rhs=warm_a[:, 0:WARMC],
            start=True, stop=True,
        )

    dx2 = [tab[:K, b * W : (b + 1) * W] for b in range(B)]
    tts = [tab[:K, 2 * W + b * DLOC : 2 * W + (b + 1) * DLOC] for b in range(B)]
    c = {"dve": 0, "act": 0, "pe": 0, "st": 0}
    rhs_done = {}

    # ---- DVE: rhs builds for (0,0), (0,1), (1,1); fused is_le per half ----
    nc.vector.wait_ge(s_in, 16)
    for b, h in ((0, 0), (0, 1), (1, 1)):
        nc.vector.tensor_tensor(
            out=rhs[b][:K, h * NW : (h + 1) * NW].rearrange("p (z x) -> p z x", x=W),
            in0=dx2[b].rearrange("p x -> p () x").to_broadcast([K, ZH, W]),
            in1=tts[b][:, h * ZH : (h + 1) * ZH]
            .rearrange("p z -> p z ()")
            .to_broadcast([K, ZH, W]),
            op=op.is_le,
        ).then_inc(s_dve, 1)
        c["dve"] += 1
        rhs_done[(b, h)] = ("dve", c["dve"])

    # ---- ACT: rhs build for (1,0) via relu(t - dx2) one z-slice at a time ----
    nc.scalar.wait_ge(s_in, 16)
    for zi in range(ZH):
        ins = nc.scalar.activation(
            out=rhs[1][:K, zi * W : (zi + 1) * W],
            in_=dx2[1],
            func=Act.Relu,
            bias=tts[1][:, zi : zi + 1],
            scale=-1.0,
        )
    ins.then_inc(s_act, 1)
    c["act"] += 1
    rhs_done[(1, 0)] = ("act", c["act"])

    oh128 = [oh[:K, b * H : b * H + 128] for b in range(B)]
    oh32 = [oh[:K, b * H + 128 : b * H + 160] for b in range(B)]
    mm_done = {}
    thr_done = {}
    nc.tensor.wait_ge(s_st, 16)   # oh landed

    def wait_pe(token):
        eng, v = token
        nc.tensor.wait_ge(s_dve if eng == "dve" else s_act, v)

    def emit_mm_pair(b, h, slot, extra_wait=None):
        pi = b * 2 + h
        wait_pe(rhs_done[(b, h)])
        if extra_wait is not None:
            wait_pe(extra_wait)
        for q in range(2):
            ins = nc.tensor.matmul(
                out=ps0[slot][:, q * 512 : q * 512 + HW2],
                lhsT=oh128[b],
                rhs=rhs[b][:K, h * NW + q * HW2 : h * NW + (q + 1) * HW2],
                start=True, stop=True,
            )
        ins.then_inc(s_pe, 1)
        c["pe"] += 1
        mm_done[pi] = c["pe"]

    def emit_mm_ps1(b, h):
        # quadrant (b,h) -> partitions b*64 + h*32 + yo, cols (zr, x)
        po = b * 64 + h * 32
        wait_pe(rhs_done[(b, h)])
        for q in range(2):
            ins = nc.tensor.matmul(
                out=ps1[po : po + 32, q * 512 : q * 512 + HW2],
                lhsT=oh32[b],
                rhs=rhs[b][:K, h * NW + q * HW2 : h * NW + (q + 1) * HW2],
                start=True, stop=True,
                tile_position=(0, po),
            )
        ins.then_inc(s_pe, 1)
        c["pe"] += 1
        mm_done[("ps1", b, h)] = c["pe"]

    def emit_thr(pi, E, slot):
        src = ps0[slot][:, :].rearrange("p (q c) -> p q c", c=512)[:, :, 0:HW2]
        dst = ob0[pi][:, :].rearrange("p (q c) -> p q c", c=HW2)
        if E == "dve":
            nc.vector.wait_ge(s_pe, mm_done[pi])
            nc.vector.tensor_scalar(
                out=dst, in0=src, scalar1=0.0, scalar2=None, op0=op.is_gt,
            ).then_inc(s_dve, 1)
            c["dve"] += 1
            thr_done[pi] = ("dve", c["dve"])
        else:
            nc.scalar.wait_ge(s_pe, mm_done[pi])
            nc.scalar.activation(out=dst, in_=src, func=Act.Sign).then_inc(s_act, 1)
            c["act"] += 1
            thr_done[pi] = ("act", c["act"])

    # PE order keeps the engine continuously busy:
    emit_mm_pair(0, 0, slot=0)
    emit_mm_pair(0, 1, slot=1)
    emit_mm_ps1(0, 0)
    emit_mm_ps1(0, 1)
    emit_thr(0, "act", slot=0)
    emit_thr(1, "act", slot=1)
    emit_mm_pair(1, 0, slot=2)
    emit_mm_pair(1, 1, slot=0, extra_wait=thr_done[0])
    emit_mm_ps1(1, 0)
    emit_mm_ps1(1, 1)
    emit_thr(2, "dve", slot=2)
    emit_thr(3, "act", slot=0)

    # ps1 threshold split across DVE (bank 0) and ACT (bank 1), packed ob1
    ps1_all = mm_done[("ps1", 1, 1)]
    nc.vector.wait_ge(s_pe, ps1_all)
    nc.vector.tensor_scalar(
        out=ob1[:, 0:HW2], in0=ps1[:, 0:HW2],
        scalar1=0.0, scalar2=None, op0=op.is_gt,
    ).then_inc(s_dve, 1)
    c["dve"] += 1
    ps1_q0 = c["dve"]
    nc.scalar.wait_ge(s_pe, ps1_all)
    nc.scalar.activation(
        out=ob1[:, HW2:NW], in_=ps1[:, 512 : 512 + HW2], func=Act.Sign
    ).then_inc(s_act, 1)
    c["act"] += 1
    ps1_q1 = c["act"]

    # ---- SP: stores chase thresholds ----
    for pi in range(4):
        eng, v = thr_done[pi]
        nc.sync.wait_ge(s_dve if eng == "dve" else s_act, v)
        nc.sync.dma_start(out=out0_d[pi, :, :], in_=ob0[pi][:, :]).then_inc(s_st, 16)
        c["st"] += 16
    nc.sync.wait_ge(s_dve, ps1_q0)
    nc.sync.wait_ge(s_act, ps1_q1)
    nc.sync.dma_start(out=out1_d[:, :], in_=ob1[:, :]).then_inc(s_st, 16)
    c["st"] += 16

    # ---- GPSIMD: wait for stores + oh-load, reset sems for re-execution ----
    nc.gpsimd.wait_ge(s_st, c["st"] + 16)
    nc.gpsimd.wait_ge(s_in, 16)
    nc.gpsimd.dma_reset(sem_range)
    nc.gpsimd.sem_clear(sem_range)

    nc.finalize()
    return nc


def _build_in_maps_fast(coords: np.ndarray, sel):
    """Host-precomputed tables per core: tab = dx2 ++ tts (f32), oh (bf16)."""
    import ml_dtypes

    jcol = np.arange(-4, 6, dtype=np.float32)
    xs = np.arange(W, dtype=np.float32)
    ys = np.arange(H, dtype=np.float32)

    in_maps = []
    for core in range(NCORES):
        zbase = core * DLOC
        zs = np.arange(zbase, zbase + DLOC, dtype=np.float32)
        tab = np.zeros((128, 2 * W + 2 * DLOC), dtype=np.float32)
        oh = np.zeros((128, 2 * H), dtype=np.float32)
        for b in range(B):
            pb = sel[core][b]
            pts = np.full((NPTS_FAST, 3), 1.0e9, dtype=np.float32)
            pts[: len(pb)] = pb
            # k-rows: (point, j)
            pz = np.repeat(pts[:, 0], J)              # (K,)
            py = np.repeat(pts[:, 1], J)
            px = np.repeat(pts[:, 2], J)
            jj = np.tile(jcol, NPTS_FAST)
            yrow = np.floor(py).astype(np.float32) + jj   # (K,)
            dyj = (yrow - py).astype(np.float32) ** 2
            cc = (np.float32(R2) - dyj).astype(np.float32)
            # tts[k, z] = cc - (z - pz)^2
            dz2 = ((zs[None, :] - pz[:, None]).astype(np.float32) ** 2).astype(
                np.float32
            )
            tts = (cc[:, None] - dz2).astype(np.float32)
            dx2 = ((xs[None, :] - px[:, None]).astype(np.float32) ** 2).astype(
                np.float32
            )
            tab[:K, b * W : (b + 1) * W] = dx2
            tab[:K, 2 * W + b * DLOC : 2 * W + (b + 1) * DLOC] = tts
            oh[:K, b * H : (b + 1) * H] = (ys[None, :] == yrow[:, None]).astype(
                np.float32
            )
        in_maps.append(
            {
                "tab": np.ascontiguousarray(tab),
                "oh": np.ascontiguousarray(oh.astype(ml_dtypes.bfloat16)),
            }
        )
    return in_maps


def _select_points(coords: np.ndarray):
    coords = np.ascontiguousarray(coords, dtype=np.float32)
    assert coords.shape == (B * P, 3)
    sel = []
    maxn = 0
    for core in range(NCORES):
        z0, z1 = core * DLOC, (core + 1) * DLOC
        per_b = []
        for b in range(B):
            cb = coords[b * P : (b + 1) * P]
            m = (cb[:, 0] > z0 - 5.001) & (cb[:, 0] < z1 + 5.001)
            pb = cb[m]
            per_b.append(pb)
            maxn = max(maxn, len(pb))
        sel.append(per_b)
    return sel, maxn


# ---------------------------------------------------------------------------
# Tile-scheduled fallback for NPTS > 12 (unchanged from v1).
# ---------------------------------------------------------------------------

def _build_program_tile(npts):
    from contextlib import ExitStack

    import concourse.mybir as mybir
    import concourse.tile as tile
    from concourse import bacc

    f32 = mybir.dt.float32
    bf16 = mybir.dt.bfloat16
    op = mybir.AluOpType

    K = npts * J                     # k-rows per batch
    NRG = (K + 127) // 128           # row groups per batch
    NB = B * NRG                     # total (b, g) blocks, stacked along free dim
    rgs = [min(128, K - 128 * g) for g in range(NRG)]  # rows per group

    nc = bacc.Bacc(trn_type="TRN2")

    pts_d = nc.dram_tensor("pts", [4, NB, 128], f32, kind="ExternalInput")
    zgb_d = nc.dram_tensor("zgridb", [128, DLOC], f32, kind="ExternalInput")
    xgb_d = nc.dram_tensor("xgridb", [128, W], f32, kind="ExternalInput")
    out_d = nc.dram_tensor("out", [B, H, DLOC, W], f32, kind="ExternalOutput")

    with tile.TileContext(nc) as tc, ExitStack() as ctx:
        const = ctx.enter_context(tc.tile_pool(name="const", bufs=1))
        setup = ctx.enter_context(tc.tile_pool(name="setup", bufs=1))
        rhsp = ctx.enter_context(tc.tile_pool(name="rhsp", bufs=4))
        outp = ctx.enter_context(tc.tile_pool(name="outp", bufs=8))
        psump = ctx.enter_context(tc.tile_pool(name="psump", bufs=4, space="PSUM"))
        warmp = ctx.enter_context(tc.tile_pool(name="warmp", bufs=1, space="PSUM"))

        xgb = const.tile([128, W], f32, name="xgb")
        nc.sync.dma_start(out=xgb[:, :], in_=xgb_d[:, :])
        zgb = const.tile([128, DLOC], f32, name="zgb")
        nc.sync.dma_start(out=zgb[:, :], in_=zgb_d[:, :])
        psb = setup.tile([128, 4 * NB], f32, name="psb")
        nc.sync.dma_start(out=psb[:, :], in_=pts_d[:, :])

        warm_a = const.tile([128, 512], bf16, name="warm_a")
        nc.vector.memset(warm_a[:, :], 0.0)
        warm_ps = warmp.tile([128, 512], f32, name="warm_ps")
        for w in range(14):
            nc.tensor.matmul(
                out=warm_ps[:, :], lhsT=warm_a[:, 0:128], rhs=warm_a[:, :],
                start=True, stop=True,
            )

        pzc = psb[:, 0 * NB : 1 * NB]
        pyc = psb[:, 1 * NB : 2 * NB]
        pxc = psb[:, 2 * NB : 3 * NB]
        jcc = psb[:, 3 * NB : 4 * NB]

        col = setup.tile([128, 4 * NB], f32, name="col")
        yfl = col[:, 0 * NB : 1 * NB]
        ypr = col[:, 1 * NB : 2 * NB]
        dyj = col[:, 2 * NB : 3 * NB]
        cc = col[:, 3 * NB : 4 * NB]
        MAGIC = float(12582912.0)  # 1.5 * 2**23
        nc.vector.tensor_scalar(
            out=yfl, in0=pyc, scalar1=-0.5, scalar2=MAGIC, op0=op.add, op1=op.add
        )
        nc.vector.tensor_scalar(
            out=yfl, in0=yfl, scalar1=MAGIC, scalar2=None, op0=op.subtract
        )
        nc.vector.tensor_tensor(out=ypr, in0=yfl, in1=jcc, op=op.add)
        nc.vector.tensor_tensor(out=dyj, in0=ypr, in1=pyc, op=op.subtract)
        nc.vector.tensor_tensor(out=dyj, in0=dyj, in1=dyj, op=op.mult)
        nc.vector.tensor_scalar(
            out=cc, in0=dyj, scalar1=-1.0, scalar2=R2, op0=op.mult, op1=op.add
        )

        dzt = setup.tile([128, NB * DLOC], f32, name="dzt")
        zgb_v = zgb[:, :].rearrange("p z -> p () z").to_broadcast([128, NB, DLOC])
        nc.vector.tensor_tensor(
            out=dzt[:, :].rearrange("p (blk z) -> p blk z", z=DLOC),
            in0=zgb_v,
            in1=pzc.rearrange("p blk -> p blk ()").to_broadcast([128, NB, DLOC]),
            op=op.subtract,
        )
        nc.vector.tensor_tensor(out=dzt[:, :], in0=dzt[:, :], in1=dzt[:, :], op=op.mult)
        tts = setup.tile([128, NB * DLOC], f32, name="tts")
        nc.vector.tensor_tensor(
            out=tts[:, :].rearrange("p (blk z) -> p blk z", z=DLOC),
            in0=cc.rearrange("p blk -> p blk ()").to_broadcast([128, NB, DLOC]),
            in1=dzt[:, :].rearrange("p (blk z) -> p blk z", z=DLOC),
            op=op.subtract,
        )

        dx2 = setup.tile([128, NB * W], f32, name="dx2")
        xgb_v = xgb[:, :].rearrange("p x -> p () x").to_broadcast([128, NB, W])
        nc.vector.tensor_tensor(
            out=dx2[:, :].rearrange("p (blk x) -> p blk x", x=W),
            in0=xgb_v,
            in1=pxc.rearrange("p blk -> p blk ()").to_broadcast([128, NB, W]),
            op=op.subtract,
        )
        nc.vector.tensor_tensor(out=dx2[:, :], in0=dx2[:, :], in1=dx2[:, :], op=op.mult)

        oh = setup.tile([128, NB * W], bf16, name="oh")
        nc.vector.tensor_tensor(
            out=oh[:, :].rearrange("p (blk x) -> p blk x", x=W),
            in0=xgb_v,
            in1=ypr.rearrange("p blk -> p blk ()").to_broadcast([128, NB, W]),
            op=op.is_equal,
        )

        def blk(b, g):
            return b * NRG + g

        NW = ZG * W  # 480
        ps1big = {
            b: psump.tile([128, NW], f32, name=f"ps1_{b}", tag=f"ps1_{b}", bufs=1)
            for b in range(B)
        }
        ob1big = {b: outp.tile([128, NW], f32, name=f"ob1_{b}", bufs=1) for b in range(B)}

        for b in range(B):
            for zg in range(NZG):
                rhs = []
                for g in range(NRG):
                    rg = rgs[g]
                    rhs_g = rhsp.tile([128, NW], bf16, name=f"rhs{g}", tag=f"rhs{g}")
                    for zi in range(ZG):
                        z = zg * ZG + zi
                        dst = rhs_g[:rg, zi * W : (zi + 1) * W]
                        src = dx2[:rg, blk(b, g) * W : blk(b, g) * W + W]
                        tcol = tts[:rg, blk(b, g) * DLOC + z : blk(b, g) * DLOC + z + 1]
                        if zi == 1:
                            nc.scalar.activation(
                                out=dst, in_=src,
                                func=mybir.ActivationFunctionType.Relu,
                                bias=tcol, scale=-1.0,
                            )
                        else:
                            nc.vector.tensor_scalar(
                                out=dst, in0=src, scalar1=tcol, scalar2=None,
                                op0=op.is_le,
                            )
                    rhs.append(rhs_g)

                ps0 = psump.tile([128, NW], f32, name="ps0", tag="ps0")
                for g in range(NRG):
                    rg = rgs[g]
                    nc.tensor.matmul(
                        out=ps0[:, :],
                        lhsT=oh[:rg, blk(b, g) * W : blk(b, g) * W + 128],
                        rhs=rhs[g][:rg, :],
                        start=(g == 0), stop=(g == NRG - 1),
                    )
                for g in range(NRG):
                    rg = rgs[g]
                    nc.tensor.matmul(
                        out=ps1big[b][32 * zg : 32 * zg + 32, :],
                        lhsT=oh[:rg, blk(b, g) * W + 128 : blk(b, g) * W + 160],
                        rhs=rhs[g][:rg, :],
                        start=(g == 0), stop=(g == NRG - 1),
                        tile_position=(0, 32 * zg),
                    )

                ob0 = outp.tile([128, NW], f32, name="ob0", tag="ob0")
                if zg % 2 == 0:
                    nc.scalar.activation(
                        out=ob0[:, :], in_=ps0[:, :],
                        func=mybir.ActivationFunctionType.Sign,
                    )
                else:
                    nc.vector.tensor_scalar(
                        out=ob0[:, :], in0=ps0[:, :], scalar1=0.0, scalar2=None,
                        op0=op.is_gt,
                    )
                zl = zg * ZG
                nc.sync.dma_start(
                    out=out_d[b, 0:128, zl : zl + ZG, :],
                    in_=ob0[:, :].rearrange("p (z x) -> p z x", z=ZG),
                )

            nc.vector.tensor_scalar(
                out=ob1big[b][:, :], in0=ps1big[b][:, :], scalar1=0.0, scalar2=None,
                op0=op.is_gt,
            )
            for zg in range(NZG):
                zl = zg * ZG
                nc.sync.dma_start(
                    out=out_d[b, 128:160, zl : zl + ZG, :],
                    in_=ob1big[b][32 * zg : 32 * zg + 32, :].rearrange(
                        "p (z x) -> p z x", z=ZG
                    ),
                )

    nc.finalize()
    return nc


def _build_in_maps_tile(coords: np.ndarray, sel, NPTS):
    K = NPTS * J
    NRG = (K + 127) // 128
    NB = B * NRG
    jcol = np.tile(np.arange(-4, 6, dtype=np.float32), NPTS)

    xgridb = np.ascontiguousarray(
        np.broadcast_to(np.arange(W, dtype=np.float32), (128, W))
    )

    in_maps = []
    for core in range(NCORES):
        zbase = core * DLOC
        zgridb = np.ascontiguousarray(
            np.broadcast_to(
                np.arange(zbase, zbase + DLOC, dtype=np.float32), (128, DLOC)
            )
        )
        pts2 = np.empty((4, NB, 128), dtype=np.float32)
        pts2[0:3] = 1.0e9
        pts2[3] = 0.0
        for b in range(B):
            pb = sel[core][b]
            krows = np.repeat(pb, J, axis=0)
            n = krows.shape[0]
            for g in range(NRG):
                lo, hi = g * 128, min((g + 1) * 128, n)
                if lo >= n:
                    break
                bk = b * NRG + g
                pts2[0, bk, 0 : hi - lo] = krows[lo:hi, 0]
                pts2[1, bk, 0 : hi - lo] = krows[lo:hi, 1]
                pts2[2, bk, 0 : hi - lo] = krows[lo:hi, 2]
                pts2[3, bk, 0 : hi - lo] = jcol[lo:hi]
        in_maps.append(
            {
                "pts": np.ascontiguousarray(pts2),
                "zgridb": zgridb,
                "xgridb": xgridb,
            }
        )
    return in_maps


def _build_in_maps(coords: np.ndarray):
    sel, maxn = _select_points(coords)
    if maxn <= NPTS_FAST:
        return NPTS_FAST, _build_in_maps_fast(coords, sel)
    NPTS = ((maxn + 3) // 4) * 4
    return NPTS, _build_in_maps_tile(coords, sel, NPTS)


def _get_program(npts):
    if npts not in _prog_cache:
        if npts <= NPTS_FAST:
            _prog_cache[npts] = _build_program_fast(npts)
        else:
            _prog_cache[npts] = _build_program_tile(npts)
    return _prog_cache[npts]


def kernel(x: np.ndarray, coords: np.ndarray) -> np.ndarray:
    from concourse.bass_utils import run_bass_kernel_spmd

    assert x.shape == (B, 4, D, H, W)
    NPTS, in_maps = _build_in_maps(coords)
    nc = _get_program(NPTS)
    res = run_bass_kernel_spmd(nc, in_maps, list(range(NCORES)))

    full = np.empty((B, 1, D, H, W), dtype=np.float32)
    for core in range(NCORES):
        zsl = slice(core * DLOC, (core + 1) * DLOC)
        r = res.results[core]
        if NPTS <= NPTS_FAST:
            # out0: [4, 128, 960] u8, pair p = b*2+h -> z in [6h,6h+6), y<128
            o0 = r["out0"].reshape(B, 2, 128, ZH, W)
            full[:, 0, zsl, 0:128, :] = (
                o0.transpose(0, 1, 3, 2, 4).reshape(B, DLOC, 128, W)
            )
            # out1: [128, 960] u8, partitions (b, h, yo), cols (zr, x)
            o1 = r["out1"].reshape(B, 2, 32, ZH, W)
            full[:, 0, zsl, 128:160, :] = (
                o1.transpose(0, 1, 3, 2, 4).reshape(B, DLOC, 32, W)
            )
        else:
            o = r["out"]  # [B, H, DLOC, W] f32
            full[:, 0, zsl] = o.transpose(0, 2, 1, 3)
    return full
